# revision 8
# baseline (speedup 1.0000x reference)
"""Trainium2 Bass kernel for nn_AttentionLayer_78632261255284 (sparse_attention).

Strategy (8-way query-row sharding, mask chain eliminated by algebra):
  The reference mask F = max(eye, .5*M2, .25*M3, .125*M4) with Mk = B^k is
  dominated everywhere by .125*M4 (>=11x, structural: the virtual node links
  to/from every node, so M2 = J + R with J all-ones). Expanding
  M4 = M2^2 = nJ + 1c^T + r1^T + R^2 and splitting R's (large) virtual
  row/col a, b out of R = e0 a^T + b e0^T - k e0 e0^T + Rt gives

    M4 = nJ + 1c^T + r1^T + b a^T + e0(Rt^T a)^T + (Rt b)e0^T
         + (a^T b - k^2) e0 e0^T + Rt^2

  where every term except Rt^2 comes from O(N) marginal vectors (host
  computes them with O(N^2) vec-mats, same class as the input formatting).
  Rt^2 (the pure length-4 path counts between real nodes) is <=1.8% of M4
  pointwise; dropping it gives 1.5e-3 end-to-end rel error (validated).

  On device F is materialized per core as a contract-4 PE matmul
  (lhsT rows {1, c, a, Rt^T a} x rhs rows {1, inv, b*inv, delta*inv}),
  normalized per query by inv[q] = 1/(n + r[q]) (cancels in softmax, keeps
  fp16 ranges tame), plus a partition-0 (virtual key) fixup vector.

  bk is dropped entirely: scores gain q . bk, constant across keys for a
  fixed query, which cancels in softmax (exact).

  Schedule: F-build + q-proj; pass1 streams X chunks computing V and
  kT for head-pairs 0-1; attention runs per head-pair with QK(ck+1)
  emitted before AV(ck) so the PE never head-blocks on exp/mult; kT for
  head-pairs 2-3 is computed between th0 and th1; each pair's softmax
  epilogue (reciprocal of the ones-column denominator, PE broadcast,
  normalize) is drained to SBUF immediately and deferred into the next
  pair's slack. exp runs on scalar (1024-wide tiles), the F multiply on
  vector (fp16 2x rate). Host adds bo at the end.

Numerics: fp16 operands, fp32 PSUM (fp8 was tested and rejected: the
output is a near-uniform attention average, so per-element quantization
noise passes through at full relative strength -> 4.8e-2 rel err).
"""

import numpy as np

import concourse.bass as bass
import concourse.mybir as mybir
import concourse.tile as tile
from concourse import bacc
from concourse.bass_utils import run_bass_kernel_spmd

P = 128
N = 4096  # nodes (+virtual)
NB = N // P  # 32 node blocks
EMB = 512
ET = EMB // P  # 4 embed blocks
HEADS = 8
HD = 64
SLAB = 512  # rows per core
NCORES = 8
AC = 2  # kb-blocks per attention exp/mult batch
NCK = NB // AC

dt = mybir.dt
AF = mybir.ActivationFunctionType
ALU = mybir.AluOpType

_NC_CACHE = {}
LAST_RESULT = None


def _install_ntff_shim():
    """Provide antenv.axon_hooks if the image lacks it, so trace=True under
    axon works (profiling via ctypes into libaxon_pjrt.so). No-op if the
    real module exists or the .so lacks the symbols."""
    try:
        from antenv.axon_hooks import get_axon_ntff_profile_hook  # noqa: F401
        return
    except ImportError:
        pass
    import contextlib
    import ctypes
    import sys
    import types

    so_path = "/opt/axon/libaxon_pjrt.so"
    hook = None
    try:
        lib = ctypes.CDLL(so_path)
        if hasattr(lib, "axon_start_nrt_profile"):
            lib.axon_start_nrt_profile.argtypes = [
                ctypes.POINTER(ctypes.c_int64),
                ctypes.c_size_t,
            ]
            lib.axon_start_nrt_profile.restype = ctypes.c_int64
            lib.axon_stop_nrt_profile.argtypes = [ctypes.c_char_p]
            lib.axon_stop_nrt_profile.restype = ctypes.c_int64

            @contextlib.contextmanager
            def _hook(output_dir, device_ids):
                import jax

                jax.devices()
                if device_ids:
                    ids = (ctypes.c_int64 * len(device_ids))(*device_ids)
                    rc = lib.axon_start_nrt_profile(ids, len(device_ids))
                else:
                    rc = lib.axon_start_nrt_profile(None, 0)
                if rc != 0:
                    raise RuntimeError(f"axon_start_nrt_profile rc={rc}")
                try:
                    yield
                finally:
                    n = lib.axon_stop_nrt_profile(str(output_dir).encode())
                    if n < 0:
                        raise RuntimeError(f"axon_stop_nrt_profile rc={n}")

            hook = _hook
    except OSError:
        pass

    mod = types.ModuleType("antenv.axon_hooks")
    mod.get_axon_ntff_profile_hook = lambda: hook
    mod.set_axon_ntff_profile_hook = lambda h: None
    sys.modules["antenv.axon_hooks"] = mod


_install_ntff_shim()


def build_bass():
    nc = bacc.Bacc("TRN2", target_bir_lowering=False, debug=False, num_devices=NCORES)

    xt = nc.dram_tensor("xt", [P, ET, N], dt.float16, kind="ExternalInput")
    xtr = nc.dram_tensor("xtr", [EMB, SLAB], dt.float16, kind="ExternalInput")
    wq = nc.dram_tensor("wq", [EMB, EMB], dt.float16, kind="ExternalInput")
    wk = nc.dram_tensor("wk", [EMB, EMB], dt.float16, kind="ExternalInput")
    wv = nc.dram_tensor("wv", [EMB, EMB], dt.float16, kind="ExternalInput")
    wo = nc.dram_tensor("wo", [EMB, EMB], dt.float16, kind="ExternalInput")
    bq = nc.dram_tensor("bq", [EMB], dt.float32, kind="ExternalInput")
    bv = nc.dram_tensor("bv", [EMB], dt.float32, kind="ExternalInput")
    fl = nc.dram_tensor("fl", [4, NB, P], dt.float16, kind="ExternalInput")
    fr = nc.dram_tensor("fr", [4, SLAB], dt.float16, kind="ExternalInput")
    fx0 = nc.dram_tensor("fx0", [1, SLAB], dt.float32, kind="ExternalInput")
    out = nc.dram_tensor("out", [SLAB, EMB], dt.float32, kind="ExternalOutput")

    with tile.TileContext(nc) as tc:
        with (
            tc.tile_pool(name="persist", bufs=1) as persist,
            tc.tile_pool(name="attn", bufs=8) as attn,
            tc.tile_pool(name="small", bufs=2) as small,
            tc.tile_pool(name="posb", bufs=4) as posb,
            tc.tile_pool(name="xstream", bufs=2) as xstream,
            tc.tile_pool(name="psA", bufs=2, space="PSUM") as psA,
            tc.tile_pool(name="psQK", bufs=2, space="PSUM") as psQK,
            tc.tile_pool(name="poR", bufs=2, space="PSUM") as poR,
        ):
            # ---------------- persistent tiles ----------------
            F = persist.tile([P, NB, SLAB], dt.float16, tag="F")
            qT = persist.tile([P, ET, SLAB], dt.float16, tag="qT")
            kT = persist.tile([P, ET, N], dt.float16, tag="kT")
            v_sb = persist.tile([P, NB, HEADS, HD + 1], dt.float16, tag="v_sb")
            out_allT = persist.tile([P, ET, SLAB], dt.float16, tag="out_allT")
            wo_sb = persist.tile([P, ET, EMB], dt.float16, tag="wo_sb")
            bv_sb = persist.tile([P, ET], dt.float32, tag="bv_sb")
            ones64 = persist.tile([1, HD], dt.float16, tag="ones64")
            fl_sb = persist.tile([4, NB, P], dt.float16, tag="fl_sb")
            fr_sb = persist.tile([4, SLAB], dt.float16, tag="fr_sb")
            fx0_sb = persist.tile([1, SLAB], dt.float32, tag="fx0_sb")
            xtr_sb = persist.tile([P, ET, SLAB], dt.float16, tag="xtr_sb")
            wq_sb = persist.tile([P, ET, EMB], dt.float16, tag="wq_sb")
            wk_sb = persist.tile([P, ET, EMB], dt.float16, tag="wk_sb")
            wv_sb = persist.tile([P, ET, EMB], dt.float16, tag="wv_sb")
            bq_sb = persist.tile([P, ET], dt.float32, tag="bq_sb")

            nc.sync.dma_start(fl_sb[:], fl[:])
            nc.sync.dma_start(fr_sb[:], fr[:])
            nc.sync.dma_start(fx0_sb[:], fx0[:])
            nc.sync.dma_start(xtr_sb[:], xtr.rearrange("(t p) q -> p t q", p=P))
            nc.sync.dma_start(wq_sb[:], wq.rearrange("(t p) c -> p t c", p=P))
            nc.sync.dma_start(bq_sb[:], bq.rearrange("(t p) -> p t", p=P))
            nc.sync.dma_start(wk_sb[:], wk.rearrange("(t p) c -> p t c", p=P))
            nc.sync.dma_start(wv_sb[:], wv.rearrange("(t p) c -> p t c", p=P))
            nc.sync.dma_start(wo_sb[:], wo.rearrange("(t p) c -> p t c", p=P))
            nc.sync.dma_start(bv_sb[:], bv.rearrange("(t p) -> p t", p=P))
            nc.vector.memset(ones64[:], 1.0)

            # ------- F = rank-4 mask build (contract-4 matmuls) -------
            for kb in range(NB):
                ps = psA.tile([P, SLAB], dt.float32, tag="acc")
                nc.tensor.matmul(
                    ps[:], fl_sb[:, kb, :], fr_sb[:], start=True, stop=True
                )
                if kb == 0:
                    # virtual-key (partition 0) fixup: (Rt b + corner)*inv
                    nc.vector.tensor_tensor(
                        out=ps[0:1, :], in0=ps[0:1, :], in1=fx0_sb[:],
                        op=ALU.add,
                    )
                if kb % 2 == 0:
                    nc.scalar.copy(F[:, kb, :], ps[:])
                else:
                    nc.vector.tensor_copy(F[:, kb, :], ps[:])

            # ---------------- q projection ----------------
            for hb in range(ET):
                ps = psA.tile([P, SLAB], dt.float32, tag="acc")
                for t in range(ET):
                    nc.tensor.matmul(
                        ps[:],
                        wq_sb[:, t, hb * P : (hb + 1) * P],
                        xtr_sb[:, t, :],
                        start=(t == 0),
                        stop=(t == ET - 1),
                    )
                nc.scalar.activation(
                    qT[:, hb, :], ps[:], AF.Identity, bias=bq_sb[:, hb : hb + 1]
                )

            def kv_pass(hbs, with_v, copy_eng):
                # stream X chunks: V (all heads) and kT for head-pairs hbs
                for nck in range(N // SLAB):
                    xc = xstream.tile([P, ET, SLAB], dt.float16, tag="xc")
                    nc.sync.dma_start(
                        xc[:], xt[:, :, nck * SLAB : (nck + 1) * SLAB]
                    )
                    if with_v:
                        for j in range(SLAB // P):
                            nb = nck * (SLAB // P) + j
                            ps = psA.tile([P, SLAB], dt.float32, tag="acc")
                            for t in range(ET):
                                nc.tensor.matmul(
                                    ps[:],
                                    xc[:, t, j * P : (j + 1) * P],
                                    wv_sb[:, t, :],
                                    start=(t == 0),
                                    stop=(t == ET - 1),
                                )
                            nc.vector.tensor_copy(
                                v_sb[:, nb, :, 0:HD],
                                ps.rearrange("p (h d) -> p h d", h=HEADS),
                            )
                    for hb in hbs:
                        ps = psA.tile([P, SLAB], dt.float32, tag="acc")
                        for t in range(ET):
                            nc.tensor.matmul(
                                ps[:],
                                wk_sb[:, t, hb * P : (hb + 1) * P],
                                xc[:, t, :],
                                start=(t == 0),
                                stop=(t == ET - 1),
                            )
                        if copy_eng == "scalar":
                            nc.scalar.copy(
                                kT[:, hb, nck * SLAB : (nck + 1) * SLAB], ps[:]
                            )
                        else:
                            nc.vector.tensor_copy(
                                kT[:, hb, nck * SLAB : (nck + 1) * SLAB], ps[:]
                            )

            kv_pass([0, 1], True, "scalar")
            nc.vector.memset(v_sb[:, :, :, HD : HD + 1], 1.0)

            # ---------------- attention ----------------
            def emit_epilogue(th, po_sbs):
                # softmax normalize from drained accumulators (SBUF)
                for sub, po in enumerate([0, HD]):
                    po_sb = po_sbs[sub]
                    row = small.tile([1, SLAB], dt.float32, tag="row")
                    rscratch = small.tile([1, SLAB], dt.float32, tag="rscratch")
                    nc.vector.tensor_copy(row[:], po_sb[HD : HD + 1, :])
                    nc.vector.reciprocal_approx_accurate(
                        row[:], row[:], rscratch[:]
                    )
                    row16 = small.tile([1, SLAB], dt.float16, tag="row16")
                    nc.vector.tensor_copy(row16[:], row[:])
                    rps = psA.tile([P, SLAB], dt.float32, tag="acc")
                    nc.tensor.matmul(
                        rps[0:HD, :], ones64[:], row16[:], start=True, stop=True
                    )
                    r_sb = small.tile([HD, SLAB], dt.float32, tag="r_sb")
                    nc.vector.tensor_copy(r_sb[:], rps[0:HD, :])
                    otmp = small.tile([HD, SLAB], dt.float32, tag="otmp")
                    nc.vector.tensor_tensor(
                        out=otmp[:], in0=po_sb[0:HD, :], in1=r_sb[:],
                        op=ALU.mult,
                    )
                    nc.vector.tensor_scalar_add(
                        out_allT[po : po + HD, th, :], otmp[:],
                        bv_sb[po : po + HD, th : th + 1],
                    )

            pending_epi = None
            for th in range(HEADS // 2):
                pos = [0, HD]
                po_a = poR.tile([P, SLAB], dt.float32, tag="po")
                po_b = poR.tile([P, SLAB], dt.float32, tag="po")
                po_tiles = [po_a, po_b]
                prev = None
                for ck in range(NCK + 1):
                    cur = None
                    if ck < NCK:
                        cur = []
                        for sub, po in enumerate(pos):
                            psq = psQK.tile([P, AC, SLAB], dt.float32, tag="psq")
                            for j in range(AC):
                                kb = ck * AC + j
                                nc.tensor.matmul(
                                    psq[:, j, :],
                                    kT[po : po + HD, th, kb * P : (kb + 1) * P],
                                    qT[po : po + HD, th, :],
                                    start=True,
                                    stop=True,
                                )
                            sexp = attn.tile([P, AC, SLAB], dt.float16, tag="sexp")
                            nc.scalar.activation(sexp[:], psq[:], AF.Exp)
                            nc.vector.tensor_tensor(
                                out=sexp[:],
                                in0=sexp[:],
                                in1=F[:, ck * AC : (ck + 1) * AC, :],
                                op=ALU.mult,
                            )
                            cur.append(sexp)
                    if ck >= 1:
                        # AV for the previous ck, emitted after this ck's QK
                        # so the PE queue never waits on exp/mult
                        for sub in range(2):
                            h = 2 * th + sub
                            for j in range(AC):
                                kb = (ck - 1) * AC + j
                                nc.tensor.matmul(
                                    po_tiles[sub][0 : HD + 1, :],
                                    v_sb[:, kb, h, :],
                                    prev[sub][:, j, :],
                                    start=(kb == 0),
                                    stop=(kb == NB - 1),
                                )
                    prev = cur
                    if ck == 2 and pending_epi is not None:
                        emit_epilogue(*pending_epi)
                        pending_epi = None

                # drain accumulators to SBUF fast so poR frees for next pair
                po_sbs = []
                for sub in range(2):
                    po_sb = posb.tile([HD + 1, SLAB], dt.float32, tag="po_sb")
                    nc.vector.tensor_copy(po_sb[:], po_tiles[sub][0 : HD + 1, :])
                    po_sbs.append(po_sb)
                pending_epi = (th, po_sbs)

                if th == 0:
                    kv_pass([2, 3], False, "scalar")

            emit_epilogue(*pending_epi)

            # ---------------- output projection ----------------
            for qb in range(ET):
                ps = psA.tile([P, SLAB], dt.float32, tag="acc")
                for t in range(ET):
                    nc.tensor.matmul(
                        ps[:],
                        out_allT[:, t, qb * P : (qb + 1) * P],
                        wo_sb[:, t, :],
                        start=(t == 0),
                        stop=(t == ET - 1),
                    )
                fin = small.tile([P, SLAB], dt.float32, tag="fin")
                nc.vector.tensor_copy(fin[:], ps[:])
                nc.sync.dma_start(out[qb * P : (qb + 1) * P, :], fin[:])

    nc.compile()
    return nc


def _prep_host(input_embeddings, edge_index, num_nodes, Wq, bq, Wk, bk, Wv, bv, Wo, bo):
    n = int(num_nodes) + 1
    assert n == N

    B = np.zeros((n, n), dtype=np.float64)
    idx = np.arange(n)
    B[idx, idx] = 1.0
    e0 = np.asarray(edge_index[0], dtype=np.int64)
    e1 = np.asarray(edge_index[1], dtype=np.int64)
    B[e0, e1] = 1.0
    B[: n - 1, n - 1] = 1.0
    B[n - 1, : n - 1] = 1.0

    # node permutation: virtual node moved to position 0
    perm = np.arange(n)
    perm[0], perm[n - 1] = n - 1, 0
    Bp = np.ascontiguousarray(B[perm][:, perm])

    # O(N^2) marginal vectors for the rank-4 M4 decomposition
    colB = Bp.sum(axis=0)
    rowB = Bp.sum(axis=1)
    a = colB - 1.0  # R[0, :]
    b = rowB - 1.0  # R[:, 0]
    kappa = float(n - 1)
    c = colB @ Bp - n  # colsums of R
    r = Bp @ rowB - n  # rowsums of R
    ab = float(a @ b)
    Rta = (a @ Bp) @ Bp - a.sum() - kappa * a
    Rta[0] += -ab + kappa * kappa  # = 0 (virtual row of Rt is zero)
    Rb = Bp @ (Bp @ b) - b.sum() - kappa * b
    Rb[0] += -ab + kappa * kappa  # = 0
    corner = ab - kappa * kappa

    X = np.asarray(input_embeddings, dtype=np.float32)[perm]
    xt = np.ascontiguousarray(X.T.astype(np.float16))
    xt_dev = np.ascontiguousarray(xt.reshape(ET, P, N).transpose(1, 0, 2))

    wq_h = np.ascontiguousarray((np.asarray(Wq, np.float32) * 0.125).astype(np.float16))
    wk_h = np.ascontiguousarray(np.asarray(Wk, np.float32).astype(np.float16))
    wv_h = np.ascontiguousarray(np.asarray(Wv, np.float32).astype(np.float16))
    wo_h = np.ascontiguousarray(np.asarray(Wo, np.float32).astype(np.float16))
    bq_h = np.ascontiguousarray(np.asarray(bq, np.float32) * 0.125)
    bv_h = np.ascontiguousarray(np.asarray(bv, np.float32))

    in_maps = []
    for core in range(NCORES):
        r0 = core * SLAB
        r_s = r[r0 : r0 + SLAB]
        b_s = b[r0 : r0 + SLAB]
        inv = 1.0 / (n + r_s)
        delta = np.zeros(SLAB)
        if core == 0:
            delta[0] = 1.0
        # per-row pow2 rebalance keeps lhsT under fp16 overflow and rhs
        # above fp16-subnormal range
        lrows = [np.ones(n), c, a, Rta]
        rrows = [np.ones(SLAB), inv, b_s * inv, delta * inv]
        for i in range(1, 4):
            while np.max(np.abs(lrows[i])) > 28000.0:
                lrows[i] = lrows[i] * 0.5
                rrows[i] = rrows[i] * 2.0
            nz = np.abs(rrows[i][rrows[i] != 0])
            while nz.size and nz.min() < 7e-5:
                lrows[i] = lrows[i] * 0.5
                rrows[i] = rrows[i] * 2.0
                nz = nz * 2.0
            assert np.min(np.abs(lrows[i][lrows[i] != 0])) >= 7e-5
        fl_h = np.ascontiguousarray(
            np.stack(lrows).astype(np.float16).reshape(4, NB, P)
        )
        fr_h = np.ascontiguousarray(np.stack(rrows).astype(np.float16))
        fx0_h = (Rb[r0 : r0 + SLAB] * inv).astype(np.float64)
        if core == 0:
            fx0_h[0] += corner * inv[0]
        xtr = np.ascontiguousarray(xt[:, r0 : r0 + SLAB])
        in_maps.append(
            {
                "xt": xt_dev,
                "xtr": xtr,
                "wq": wq_h,
                "wk": wk_h,
                "wv": wv_h,
                "wo": wo_h,
                "bq": bq_h,
                "bv": bv_h,
                "fl": fl_h,
                "fr": fr_h,
                "fx0": np.ascontiguousarray(
                    fx0_h.astype(np.float32).reshape(1, SLAB)
                ),
            }
        )
    return in_maps


def kernel(**inputs) -> np.ndarray:
    if "nc" not in _NC_CACHE:
        _NC_CACHE["nc"] = build_bass()
    nc = _NC_CACHE["nc"]

    in_maps = _prep_host(**inputs)
    res = run_bass_kernel_spmd(nc, in_maps, core_ids=list(range(NCORES)))
    global LAST_RESULT
    LAST_RESULT = res
    bo = np.asarray(inputs["bo"], dtype=np.float32)
    slabs = [res.results[c]["out"] for c in range(NCORES)]
    dev_out = np.concatenate(slabs, axis=0)
    # undo the virtual-node-to-front permutation (device row i = node perm[i])
    perm = np.arange(N)
    perm[0], perm[N - 1] = N - 1, 0
    full = np.empty_like(dev_out)
    full[perm] = dev_out
    return (full + bo[None, :]).astype(np.float32)


if __name__ == "__main__":
    import reference

    inputs = {k: np.asarray(v) if not np.isscalar(v) else v for k, v in reference.setup_inputs().items()}
    got = kernel(**inputs)
    print("kernel output:", got.shape, got.dtype)
